# revision 7
# baseline (speedup 1.0000x reference)
"""Trainium2 Bass kernel for the KGEncoder RGCN (nn_KGEncoder_14027363188782).

Math (per batch element b, L=5 layers):
    x0 = ent_emb                                             (E, D)
    per layer i:
      y_r   = x @ Wb_x[i,r] + 1 * c[i,r]^T    (E, NB)  where c[i,r] = rel_r @ Wb_rel[i,r]
      Z     = sum_r adj_r @ y_r               (E, NB)  == sup @ Wb[i]  (deg term folded via c)
      h     = relu(Z @ Ww[i] + bias[i])
      g     = sigmoid(h @ Wh[i] + bh[i])
      x     = x + g * (h - x)
    out_b = sum_e x[e] * m[e] / max(sum_e m[e], 1)

Sharding: core c handles b = c // 2 (pair-replicated, no collectives).
adj is shipped pre-transposed (j-major) in bf16 (exact for 0/1 values).
Big matmul: out Z.T (NB x E) = sum_{r,k} y'[kchunk]_r.T @ adjT_r[kchunk];
NRES relations stay resident in SBUF, the rest stream from HBM each layer.
"""

import numpy as np
import ml_dtypes

import concourse.bacc as bacc
import concourse.bass as bass
import concourse.mybir as mybir
import concourse.tile as tile
from concourse import bass_utils
from concourse.bass import MemorySpace

B, R, E, D, HID, L, NB = 4, 10, 1500, 100, 100, 5, 3
EP = 1536           # entity (j) dim padded to 12*128
CH = EP // 128      # 12 k-chunks
NRES = 4            # relations resident in SBUF
SG = 3              # k-chunks per streamed stage tile
NW = 500            # psum free-dim chunk (3 per row of E)
RNB = R * NB        # 30
f32 = mybir.dt.float32
bf16 = mybir.dt.bfloat16
AF = mybir.ActivationFunctionType
AX = mybir.AxisListType

_NC_CACHE = {}


def _build_nc():
    nc = bacc.Bacc("TRN2", target_bir_lowering=False, debug=False)

    adjT = nc.dram_tensor("adjT", [R, EP, E], bf16, kind="ExternalInput").ap()
    xT0 = nc.dram_tensor("xT0", [D, E], f32, kind="ExternalInput").ap()
    maskrep = nc.dram_tensor("maskrep", [HID, E], f32, kind="ExternalInput").ap()
    relT = nc.dram_tensor("relT", [D, R], f32, kind="ExternalInput").ap()
    wbxD = nc.dram_tensor("wbx", [L, D, RNB], f32, kind="ExternalInput").ap()
    wbrD = nc.dram_tensor("wbr", [L, D, RNB], f32, kind="ExternalInput").ap()
    wwD = nc.dram_tensor("ww", [L, NB, HID], f32, kind="ExternalInput").ap()
    whD = nc.dram_tensor("wh", [L, HID, HID], f32, kind="ExternalInput").ap()
    biasD = nc.dram_tensor("biasL", [L, HID], f32, kind="ExternalInput").ap()
    bhD = nc.dram_tensor("bhL", [L, HID], f32, kind="ExternalInput").ap()
    graphD = nc.dram_tensor("graph", [HID, 1], f32, kind="ExternalOutput").ap()

    with tile.TileContext(nc) as tc:
        with (
            tc.tile_pool(name="singles", bufs=1) as singles,
            tc.tile_pool(name="resp", bufs=1) as resp,
            tc.tile_pool(name="stagep", bufs=4) as stagep,
            tc.tile_pool(name="ypool", bufs=2) as ypool,
            tc.tile_pool(name="workp", bufs=2) as workp,
            tc.tile_pool(name="psY", bufs=2, space=MemorySpace.PSUM) as psY,
            tc.tile_pool(name="psC", bufs=1, space=MemorySpace.PSUM) as psC,
            tc.tile_pool(name="psB", bufs=1, space=MemorySpace.PSUM) as psB,
        ):
            # ---- persistent state ----
            xT = singles.tile([D, EP], f32, tag="xT", name="xT")
            nc.sync.dma_start(out=xT[:, 0:E], in_=xT0)
            nc.vector.memset(xT[:, E:EP], 0.0)

            ones = singles.tile([1, 128], f32, tag="ones", name="ones")
            nc.vector.memset(ones[:, :], 1.0)

            mask_sb = singles.tile([HID, E], f32, tag="mask", name="mask_sb")
            nc.sync.dma_start(out=mask_sb[:, :], in_=maskrep)

            relT_sb = singles.tile([D, R], f32, tag="relT", name="relT_sb")
            nc.sync.dma_start(out=relT_sb[:, :], in_=relT)

            wbx_sb, wbr_sb, ww_sb, wh_sb, bias_sb, bh_sb = [], [], [], [], [], []
            for i in range(L):
                wx = singles.tile([D, RNB], f32, tag=f"wbx{i}", name=f"wbx{i}")
                nc.sync.dma_start(out=wx[:, :], in_=wbxD[i])
                wbx_sb.append(wx)
                wr = singles.tile([D, RNB], f32, tag=f"wbr{i}", name=f"wbr{i}")
                nc.sync.dma_start(out=wr[:, :], in_=wbrD[i])
                wbr_sb.append(wr)
                wwt = singles.tile([NB, HID], f32, tag=f"ww{i}", name=f"ww{i}")
                nc.sync.dma_start(out=wwt[:, :], in_=wwD[i])
                ww_sb.append(wwt)
                wht = singles.tile([HID, HID], f32, tag=f"wh{i}", name=f"wh{i}")
                nc.sync.dma_start(out=wht[:, :], in_=whD[i])
                wh_sb.append(wht)
                bt = singles.tile([HID, 1], f32, tag=f"bias{i}", name=f"bias{i}")
                nc.sync.dma_start(out=bt[:, :], in_=biasD[i].unsqueeze(1))
                bias_sb.append(bt)
                bht = singles.tile([HID, 1], f32, tag=f"bh{i}", name=f"bh{i}")
                nc.sync.dma_start(out=bht[:, :], in_=bhD[i].unsqueeze(1))
                bh_sb.append(bht)

            # resident adjT relations: tile (128, CH*E), chunk k at cols [k*E, (k+1)*E)
            res_tiles = []
            for r in range(NRES):
                rt = resp.tile([128, CH * E], bf16, tag=f"res{r}", name=f"res{r}")
                nc.sync.dma_start(
                    out=rt[:, :].rearrange("p (k i) -> p k i", k=CH),
                    in_=adjT[r].rearrange("(k p) i -> p k i", p=128),
                )
                res_tiles.append(rt)

            # ---- layers ----
            for i in range(L):
                # c[i, r, :] = rel_r @ Wb_rel[i, r]   -> psum row 0, cols 3r..3r+3
                psc = psC.tile([1, RNB], f32, tag="c", name=f"psc{i}")
                for r in range(R):
                    nc.tensor.matmul(
                        psc[:, 3 * r : 3 * r + 3],
                        relT_sb[:, r : r + 1],
                        wbr_sb[i][:, 3 * r : 3 * r + 3],
                        start=True, stop=True,
                    )
                c_sb = workp.tile([1, RNB], f32, tag="c_sb", name=f"c_sb{i}", bufs=2)
                nc.scalar.copy(out=c_sb[:, :], in_=psc[:, :])

                # y'[kchunk] = x[kchunk] @ Wbx[i]  + 1 (x) c   -> bf16 (128, RNB) per chunk
                y_all = ypool.tile([128, CH * RNB], bf16, tag="y_all", name=f"y_all{i}")
                for k in range(CH):
                    psy = psY.tile([128, RNB], f32, tag="y", name=f"psy{i}_{k}")
                    nc.tensor.matmul(
                        psy[:, :], xT[:, k * 128 : (k + 1) * 128], wbx_sb[i][:, :],
                        start=True, stop=False,
                    )
                    nc.tensor.matmul(
                        psy[:, :], ones[:, :], c_sb[:, :],
                        start=False, stop=True,
                    )
                    nc.scalar.copy(out=y_all[:, k * RNB : (k + 1) * RNB], in_=psy[:, :])

                # Z.T (NB, E) = sum_{r, k} y'_r[k].T @ adjT_r[k]
                psz = psB.tile([NB, 3, 512], f32, tag="big", name=f"psz{i}")
                total_per_n = R * CH
                cnt = 0

                def bigmm(rhs_chunk, r, k):
                    nonlocal cnt
                    lhs = y_all[:, k * RNB + 3 * r : k * RNB + 3 * r + 3]
                    for n in range(3):
                        nc.tensor.matmul(
                            psz[:, n, 0:NW],
                            lhs,
                            rhs_chunk[:, n * NW : (n + 1) * NW],
                            start=(cnt == 0), stop=(cnt == total_per_n - 1),
                        )
                    cnt += 1

                for r in range(NRES):
                    for k in range(CH):
                        bigmm(res_tiles[r][:, k * E : (k + 1) * E], r, k)
                for r in range(NRES, R):
                    for g in range(CH // SG):
                        stg = stagep.tile([128, SG * E], bf16, tag="stage",
                                          name=f"stg{i}_{r}_{g}")
                        nc.sync.dma_start(
                            out=stg[:, :].rearrange("p (k i) -> p k i", k=SG),
                            in_=adjT[r, g * SG * 128 : (g + 1) * SG * 128, :]
                                .rearrange("(k p) i -> p k i", p=128),
                        )
                        for kk in range(SG):
                            bigmm(stg[:, kk * E : (kk + 1) * E], r, g * SG + kk)
                assert cnt == total_per_n

                z_sb = workp.tile([NB, E], f32, tag="z_sb", name=f"z_sb{i}", bufs=1)
                nc.scalar.copy(
                    out=z_sb[:, :].rearrange("p (a b) -> p a b", a=3),
                    in_=psz[:, :, 0:NW],
                )

                # h.T = relu(Ww.T @ Z.T + bias)
                psh = psB.tile([HID, 3, 512], f32, tag="big", name=f"psh{i}")
                for n in range(3):
                    nc.tensor.matmul(
                        psh[:, n, 0:NW], ww_sb[i][:, :],
                        z_sb[:, n * NW : (n + 1) * NW],
                        start=True, stop=True,
                    )
                h_sb = workp.tile([HID, E], f32, tag="h", name=f"h_sb{i}", bufs=1)
                nc.scalar.activation(
                    h_sb[:, :].rearrange("p (a b) -> p a b", a=3),
                    psh[:, :, 0:NW],
                    AF.Relu, bias=bias_sb[i][:, :],
                )

                # g.T = sigmoid(Wh.T @ h.T + bh)   (kept in PSUM)
                psg = psB.tile([HID, 3, 512], f32, tag="big", name=f"psg{i}")
                for n in range(3):
                    nc.tensor.matmul(
                        psg[:, n, 0:NW], wh_sb[i][:, :],
                        h_sb[:, n * NW : (n + 1) * NW],
                        start=True, stop=True,
                    )
                nc.scalar.activation(
                    psg[:, :, 0:NW], psg[:, :, 0:NW],
                    AF.Sigmoid, bias=bh_sb[i][:, :],
                )

                # x = x + g * (h - x)
                nc.vector.tensor_sub(h_sb[:, :], h_sb[:, :], xT[:, 0:E])
                nc.vector.tensor_mul(
                    h_sb[:, :].rearrange("p (a b) -> p a b", a=3),
                    h_sb[:, :].rearrange("p (a b) -> p a b", a=3),
                    psg[:, :, 0:NW],
                )
                nc.vector.tensor_add(xT[:, 0:E], xT[:, 0:E], h_sb[:, :])

            # ---- masked mean over entities ----
            xm = workp.tile([HID, E], f32, tag="h", name="xm", bufs=1)
            nc.vector.tensor_mul(xm[:, :], xT[:, 0:E], mask_sb[:, :])
            gsum = workp.tile([HID, 1], f32, tag="gsum", name="gsum", bufs=1)
            nc.vector.reduce_sum(gsum[:, :], xm[:, :], axis=AX.X)
            den = workp.tile([HID, 1], f32, tag="den", name="den", bufs=1)
            nc.vector.reduce_sum(den[:, :], mask_sb[:, :], axis=AX.X)
            nc.vector.tensor_scalar_max(den[:, :], den[:, :], 1.0)
            nc.vector.reciprocal(den[:, :], den[:, :])
            nc.vector.tensor_mul(gsum[:, :], gsum[:, :], den[:, :])
            nc.sync.dma_start(out=graphD, in_=gsum[:, :])

    nc.compile()
    return nc


def get_nc():
    if "nc" not in _NC_CACHE:
        _NC_CACHE["nc"] = _build_nc()
    return _NC_CACHE["nc"]


def make_in_maps(adj, mask_ids, ent_emb, rel_emb, Wb, Ww, bias, Wh, bh):
    bf = ml_dtypes.bfloat16
    adj = np.asarray(adj, dtype=np.float32)
    adjT = np.zeros((B, R, EP, E), dtype=bf)
    adjT[:, :, :E, :] = adj.transpose(0, 1, 3, 2).astype(bf)
    entT = np.ascontiguousarray(np.asarray(ent_emb, np.float32).T)
    relTh = np.ascontiguousarray(np.asarray(rel_emb, np.float32).T)
    Wb5 = np.asarray(Wb, np.float32).reshape(L, R, 2, D, NB)
    wbx = np.ascontiguousarray(Wb5[:, :, 0].transpose(0, 2, 1, 3).reshape(L, D, RNB))
    wbr = np.ascontiguousarray(Wb5[:, :, 1].transpose(0, 2, 1, 3).reshape(L, D, RNB))
    maskf = np.asarray(mask_ids).astype(np.float32)
    common = dict(
        xT0=entT, relT=relTh, wbx=wbx, wbr=wbr,
        ww=np.ascontiguousarray(np.asarray(Ww, np.float32)),
        wh=np.ascontiguousarray(np.asarray(Wh, np.float32)),
        biasL=np.ascontiguousarray(np.asarray(bias, np.float32)),
        bhL=np.ascontiguousarray(np.asarray(bh, np.float32)),
    )
    in_maps = []
    for c in range(8):
        b = c // 2
        m = dict(common)
        m["adjT"] = np.ascontiguousarray(adjT[b])
        m["maskrep"] = np.ascontiguousarray(
            np.broadcast_to(maskf[b][None, :], (HID, E))
        )
        in_maps.append(m)
    return in_maps


def run(inputs, trace=False):
    nc = get_nc()
    in_maps = make_in_maps(**{k: np.asarray(v) for k, v in inputs.items()})
    res = bass_utils.run_bass_kernel_spmd(
        nc, in_maps, core_ids=list(range(8)), trace=trace
    )
    out = np.stack(
        [np.asarray(res.results[2 * b]["graph"]).reshape(HID) for b in range(B)]
    ).astype(np.float32)
    return out, res


def kernel(**inputs):
    out, _ = run(inputs, trace=False)
    return out


# revision 21
# speedup vs baseline: 141.0523x; 141.0523x over previous
"""Trainium2 Bass kernel for the KGEncoder RGCN (nn_KGEncoder_14027363188782).

Math (per batch element b, L=5 layers):
    x0 = ent_emb                                             (E, D)
    per layer i:
      y_r   = x @ Wb_x[i,r] + 1 * c[i,r]^T    (E, NB)  where c[i,r] = rel_r @ Wb_rel[i,r]
      Z     = sum_r adj_r @ y_r               (E, NB)  == sup @ Wb[i]  (deg term folded via c)
      h     = relu(Z @ Ww[i] + bias[i])
      g     = sigmoid(h @ Wh[i] + bh[i])
      x     = x + g * (h - x)
    out_b = sum_e x[e] * m[e] / max(sum_e m[e], 1)

Sharding: core c handles b = c // 2 (pair-replicated, no collectives).
adj is shipped pre-transposed (j-major) in bf16 (exact for 0/1 values).
Big matmul: out Z.T (NB x E) = sum_{r,k} y'[kchunk]_r.T @ adjT_r[kchunk];
NRES relations stay resident in SBUF, the rest stream from HBM each layer.
"""

import numpy as np
import ml_dtypes

import concourse.bacc as bacc
import concourse.bass as bass
import concourse.mybir as mybir
import concourse.tile as tile
from concourse import bass_utils
from concourse.bass import MemorySpace

B, R, E, D, HID, L, NB = 4, 10, 1500, 100, 100, 5, 3
EP = 1536           # entity (j) dim padded to 12*128
CH = EP // 128      # 12 k-chunks
FP8 = True          # fp8 adj (exact for 0/1) -> all relations SBUF-resident
DR = True           # DoubleRow fp8 matmul: 256-deep contraction, 2 elem/lane/cyc
C2 = 6              # 256-row contraction chunks (DoubleRow)
E2 = 1504           # i dim padded to 16-aligned for DoubleRow strides
YQ = 32             # y_all per-chunk col stride (16-aligned)
NRES = 10 if FP8 else 4   # relations resident in SBUF
SG = 3              # k-chunks per streamed stage tile
NW = 500            # psum free-dim chunk (3 per row of E)
RNB = R * NB        # 30
f32 = mybir.dt.float32
bf16 = mybir.dt.bfloat16
ADT = mybir.dt.float8e4 if FP8 else mybir.dt.bfloat16
ADT_NP = ml_dtypes.float8_e4m3fn if FP8 else ml_dtypes.bfloat16
AF = mybir.ActivationFunctionType
AX = mybir.AxisListType

_NC_CACHE = {}


def _build_nc():
    nc = bacc.Bacc("TRN2", target_bir_lowering=False, debug=False)

    if DR:
        adjT = nc.dram_tensor(
            "adjT", [R, C2, 128, 2, E2], ADT, kind="ExternalInput"
        ).ap()
    else:
        adjT = nc.dram_tensor("adjT", [R, EP, E], ADT, kind="ExternalInput").ap()
    xT0 = nc.dram_tensor("xT0", [D, E], f32, kind="ExternalInput").ap()
    maskrep = nc.dram_tensor("maskrep", [HID, E], f32, kind="ExternalInput").ap()
    relT = nc.dram_tensor("relT", [D, R], f32, kind="ExternalInput").ap()
    wbxD = nc.dram_tensor("wbx", [L, D, RNB], f32, kind="ExternalInput").ap()
    wbrD = nc.dram_tensor("wbr", [L, D, RNB], f32, kind="ExternalInput").ap()
    wwD = nc.dram_tensor("ww", [L, NB, HID], f32, kind="ExternalInput").ap()
    whD = nc.dram_tensor("wh", [L, HID, HID], f32, kind="ExternalInput").ap()
    biasD = nc.dram_tensor("biasL", [L, HID], f32, kind="ExternalInput").ap()
    bhD = nc.dram_tensor("bhL", [L, HID], f32, kind="ExternalInput").ap()
    graphD = nc.dram_tensor("graph", [HID, 1], f32, kind="ExternalOutput").ap()

    with tile.TileContext(nc) as tc:
        with (
            tc.tile_pool(name="singles", bufs=1) as singles,
            tc.tile_pool(name="resp", bufs=1) as resp,
            tc.tile_pool(name="stagep", bufs=4) as stagep,
            tc.tile_pool(name="ypool", bufs=2) as ypool,
            tc.tile_pool(name="workp", bufs=2) as workp,
            tc.tile_pool(name="psY", bufs=1, space=MemorySpace.PSUM) as psY,
            tc.tile_pool(name="psC", bufs=1, space=MemorySpace.PSUM) as psC,
            tc.tile_pool(name="psB", bufs=1, space=MemorySpace.PSUM) as psB,
        ):
            # ---- persistent state ----
            xT = singles.tile([D, EP], f32, tag="xT", name="xT")
            nc.sync.dma_start(out=xT[:, 0:E], in_=xT0)
            nc.vector.memset(xT[:, E:EP], 0.0)

            ones = singles.tile([1, 128], f32, tag="ones", name="ones")
            nc.vector.memset(ones[:, :], 1.0)

            mask_sb = singles.tile([HID, E], f32, tag="mask", name="mask_sb")
            nc.sync.dma_start(out=mask_sb[:, :], in_=maskrep)

            relT_sb = singles.tile([D, R], f32, tag="relT", name="relT_sb")
            nc.sync.dma_start(out=relT_sb[:, :], in_=relT)

            wbx_sb, wbr_sb, ww_sb, wh_sb, bias_sb, bh_sb = [], [], [], [], [], []
            for i in range(L):
                wx = singles.tile([D, RNB], f32, tag=f"wbx{i}", name=f"wbx{i}")
                nc.sync.dma_start(out=wx[:, :], in_=wbxD[i])
                wbx_sb.append(wx)
                wr = singles.tile([D, RNB], f32, tag=f"wbr{i}", name=f"wbr{i}")
                nc.sync.dma_start(out=wr[:, :], in_=wbrD[i])
                wbr_sb.append(wr)
                wwt = singles.tile([NB, HID], f32, tag=f"ww{i}", name=f"ww{i}")
                nc.sync.dma_start(out=wwt[:, :], in_=wwD[i])
                ww_sb.append(wwt)
                wht = singles.tile([HID, HID], f32, tag=f"wh{i}", name=f"wh{i}")
                nc.sync.dma_start(out=wht[:, :], in_=whD[i])
                wh_sb.append(wht)
                bt = singles.tile([HID, 1], f32, tag=f"bias{i}", name=f"bias{i}")
                nc.sync.dma_start(out=bt[:, :], in_=biasD[i].unsqueeze(1))
                bias_sb.append(bt)
                bht = singles.tile([HID, 1], f32, tag=f"bh{i}", name=f"bh{i}")
                nc.sync.dma_start(out=bht[:, :], in_=bhD[i].unsqueeze(1))
                bh_sb.append(bht)

            # resident adjT relations: tile (128, CH*E), chunk k at cols [k*E, (k+1)*E)
            res_tiles = []
            for r in range(NRES):
                if DR:
                    rt = resp.tile([128, C2 * 2 * E2], ADT,
                                   tag=f"res{r}", name=f"res{r}")
                    nc.sync.dma_start(
                        out=rt[:, :].rearrange("p (c t i) -> p c t i", c=C2, t=2),
                        in_=adjT[r].rearrange("c p t i -> p c t i"),
                    )
                else:
                    rt = resp.tile([128, CH * E], ADT, tag=f"res{r}", name=f"res{r}")
                    nc.sync.dma_start(
                        out=rt[:, :].rearrange("p (k i) -> p k i", k=CH),
                        in_=adjT[r].rearrange("(k p) i -> p k i", p=128),
                    )
                res_tiles.append(rt)

            # ---- layers ----
            for i in range(L):
                # c[i, r, :] = rel_r @ Wb_rel[i, r]   -> psum row 0, cols 3r..3r+3
                psc = psC.tile([1, RNB], f32, tag="c", name=f"psc{i}")
                for r in range(R):
                    nc.tensor.matmul(
                        psc[:, 3 * r : 3 * r + 3],
                        relT_sb[:, r : r + 1],
                        wbr_sb[i][:, 3 * r : 3 * r + 3],
                        start=True, stop=True,
                    )
                c_sb = workp.tile([1, RNB], f32, tag="c_sb", name=f"c_sb{i}", bufs=2)
                nc.scalar.copy(out=c_sb[:, :], in_=psc[:, :])

                # y'[kchunk] = x[kchunk] @ Wbx[i]  + 1 (x) c   -> bf16 (128, RNB) per chunk
                YS = YQ if DR else RNB
                y_all = ypool.tile([128, CH * YS], ADT, tag="y_all", name=f"y_all{i}")
                for k in range(CH):
                    psy = psY.tile([128, RNB], f32, tag="y", name=f"psy{i}_{k}")
                    nc.tensor.matmul(
                        psy[:, :], xT[:, k * 128 : (k + 1) * 128], wbx_sb[i][:, :],
                        start=True, stop=False,
                    )
                    nc.tensor.matmul(
                        psy[:, :], ones[:, :], c_sb[:, :],
                        start=False, stop=True,
                    )
                    nc.scalar.copy(out=y_all[:, k * YS : k * YS + RNB], in_=psy[:, :])

                # Z.T (NB, E) = sum_{r, k} y'_r[k].T @ adjT_r[k]
                # per i-chunk n: accumulate Z chunk, then basis/highway tail on
                # ACT/DVE overlaps the next chunk's PE matmuls
                assert DR
                h_sb = workp.tile([HID, E], f32, tag="h", name=f"h_sb{i}", bufs=1)
                y_view = y_all[:, :].rearrange("p (k q) -> p k q", q=YQ)
                res_views = [
                    res_tiles[r][:, :].rearrange("p (c t i) -> p c t i", c=C2, t=2)
                    for r in range(R)
                ]
                for n in range(3):
                    ns = slice(n * NW, (n + 1) * NW)
                    psz = psB.tile([NB, 512], f32, tag="zz", bufs=2,
                                   name=f"psz{i}_{n}")
                    cnt = 0
                    for r in range(R):
                        for c in range(C2):
                            nc.tensor.matmul(
                                psz[:, 0:NW],
                                y_view[:, 2 * c : 2 * c + 2, 3 * r : 3 * r + 3],
                                res_views[r][:, c, :, ns],
                                start=(cnt == 0),
                                stop=(cnt == R * C2 - 1),
                                perf_mode=mybir.MatmulPerfMode.DoubleRow,
                            )
                            cnt += 1
                    z_sb = workp.tile([NB, NW], f32, tag="z_sb", bufs=2,
                                      name=f"z_sb{i}_{n}")
                    nc.scalar.copy(out=z_sb[:, :], in_=psz[:, 0:NW])
                    psh = psB.tile([HID, 512], f32, tag="hh", bufs=1,
                                   name=f"psh{i}_{n}")
                    nc.tensor.matmul(
                        psh[:, 0:NW], ww_sb[i][:, :], z_sb[:, :],
                        start=True, stop=True,
                    )
                    nc.scalar.activation(
                        h_sb[:, ns], psh[:, 0:NW], AF.Relu, bias=bias_sb[i][:, :],
                    )
                    psg = psB.tile([HID, 512], f32, tag="gg", bufs=1,
                                   name=f"psg{i}_{n}")
                    nc.tensor.matmul(
                        psg[:, 0:NW], wh_sb[i][:, :], h_sb[:, ns],
                        start=True, stop=True,
                    )
                    nc.scalar.activation(
                        psg[:, 0:NW], psg[:, 0:NW], AF.Sigmoid, bias=bh_sb[i][:, :],
                    )
                    # x = x + g * (h - x)  (chunk n)
                    nc.vector.tensor_sub(h_sb[:, ns], h_sb[:, ns], xT[:, ns])
                    nc.vector.tensor_mul(h_sb[:, ns], h_sb[:, ns], psg[:, 0:NW])
                    nc.vector.tensor_add(xT[:, ns], xT[:, ns], h_sb[:, ns])

            # ---- masked mean over entities ----
            xm = workp.tile([HID, E], f32, tag="h", name="xm", bufs=1)
            nc.vector.tensor_mul(xm[:, :], xT[:, 0:E], mask_sb[:, :])
            gsum = workp.tile([HID, 1], f32, tag="gsum", name="gsum", bufs=1)
            nc.vector.reduce_sum(gsum[:, :], xm[:, :], axis=AX.X)
            den = workp.tile([HID, 1], f32, tag="den", name="den", bufs=1)
            nc.vector.reduce_sum(den[:, :], mask_sb[:, :], axis=AX.X)
            nc.vector.tensor_scalar_max(den[:, :], den[:, :], 1.0)
            nc.vector.reciprocal(den[:, :], den[:, :])
            nc.vector.tensor_mul(gsum[:, :], gsum[:, :], den[:, :])
            nc.sync.dma_start(out=graphD, in_=gsum[:, :])

    nc.compile()
    return nc


def get_nc():
    if "nc" not in _NC_CACHE:
        _NC_CACHE["nc"] = _build_nc()
    return _NC_CACHE["nc"]


def make_in_maps(adj, mask_ids, ent_emb, rel_emb, Wb, Ww, bias, Wh, bh):
    adj = np.asarray(adj, dtype=np.float32)
    if DR:
        pad = np.zeros((B, R, EP, E2), dtype=ADT_NP)
        pad[:, :, :E, :E] = adj.transpose(0, 1, 3, 2).astype(ADT_NP)
        # [b, r, c, p, t, i] = adj[b, r, i, j = c*256 + t*128 + p]
        adjT = np.ascontiguousarray(
            pad.reshape(B, R, C2, 2, 128, E2).transpose(0, 1, 2, 4, 3, 5)
        )
    else:
        adjT = np.zeros((B, R, EP, E), dtype=ADT_NP)
        adjT[:, :, :E, :] = adj.transpose(0, 1, 3, 2).astype(ADT_NP)
    entT = np.ascontiguousarray(np.asarray(ent_emb, np.float32).T)
    relTh = np.ascontiguousarray(np.asarray(rel_emb, np.float32).T)
    Wb5 = np.asarray(Wb, np.float32).reshape(L, R, 2, D, NB)
    wbx = np.ascontiguousarray(Wb5[:, :, 0].transpose(0, 2, 1, 3).reshape(L, D, RNB))
    wbr = np.ascontiguousarray(Wb5[:, :, 1].transpose(0, 2, 1, 3).reshape(L, D, RNB))
    maskf = np.asarray(mask_ids).astype(np.float32)
    common = dict(
        xT0=entT, relT=relTh, wbx=wbx, wbr=wbr,
        ww=np.ascontiguousarray(np.asarray(Ww, np.float32)),
        wh=np.ascontiguousarray(np.asarray(Wh, np.float32)),
        biasL=np.ascontiguousarray(np.asarray(bias, np.float32)),
        bhL=np.ascontiguousarray(np.asarray(bh, np.float32)),
    )
    in_maps = []
    for c in range(8):
        b = c // 2
        m = dict(common)
        m["adjT"] = np.ascontiguousarray(adjT[b])
        m["maskrep"] = np.ascontiguousarray(
            np.broadcast_to(maskf[b][None, :], (HID, E))
        )
        in_maps.append(m)
    return in_maps


def run(inputs, trace=False):
    nc = get_nc()
    in_maps = make_in_maps(**{k: np.asarray(v) for k, v in inputs.items()})
    res = bass_utils.run_bass_kernel_spmd(
        nc, in_maps, core_ids=list(range(8)), trace=trace
    )
    out = np.stack(
        [np.asarray(res.results[2 * b]["graph"]).reshape(HID) for b in range(B)]
    ).astype(np.float32)
    return out, res


def kernel(**inputs):
    out, _ = run(inputs, trace=False)
    return out
